# revision 1
# baseline (speedup 1.0000x reference)
"""Trainium2 Bass kernel for LoRA-fused QKV + RoPE + GQA causal attention + o_proj.

Problem (hardcoded): B=2, S=2048, H=2048, NH=16, KVH=4, HD=128, R=16.

Sharding: 8 cores = batch(2) x kv-head-group(4). Core c handles batch b=c//4,
kv head g=c%4 (q heads 4g..4g+3). Each core computes its 4 heads' attention and
a partial o_proj ([S,H] partial over its 512 o-dims); host sums 4 partials per
batch.

Everything on-device runs in "transposed space": projections produce qT/kT/vT
[d, s] directly (PE matmul with contraction h on partitions; host pre-transposes
x and weights), scoresT [ks, qs] feeds AV without any on-device attn transpose,
and o_proj consumes out_hT [d, s] as the stationary operand. Matmuls use fp32r
(full-rate fp32, ~1.4e-4 matmul rel err; end-to-end ~2e-4).

Softmax: no max-subtraction (scores are O(5); exp is safe in fp32), column sums
over the partition (ks) axis via an all-ones stationary matmul accumulated in
PSUM, normalization entirely on DVE (reciprocal_approx_fast + multiply) so the
in-order PE queue never blocks on it. The additive mask is applied
multiplicatively as exp(mask): SKIP tiles (exp(mask)==0) are dropped from the
schedule, all-ones tiles skip the multiply, and for an exactly-causal mask the
four distinct diagonal-tile patterns are generated on-device (no mask DMA).

Single fused loop over the 4 s-chunks: projections+RoPE -> previous chunk's
o_proj -> attention, everything pipelined; outputs stream back per chunk.
"""

import hashlib
import numpy as np

import concourse.bass as bass
import concourse.mybir as mybir
import concourse.tile as tile
from concourse import bacc
from concourse.bass_utils import run_bass_kernel_spmd

B, S, H = 2, 2048, 2048
NH, KVH, HD = 16, 4, 128
R = 16
LORA_SCALE = 32.0 / 16.0
ATTN_SCALE = HD ** -0.5

NCORES = 8
GQ = NH // KVH          # 4 q heads per core
NT = GQ + 2             # 6 projection tiles: 4 q heads, 1 k, 1 v
QD = GQ * HD            # 512
CH = 512                # s-chunk width (matmul moving-dim max for fp32)
NCH = S // CH           # 4 s-chunks
KT = H // 128           # 16 contraction k-tiles
NKS = S // 128          # 16 ks tiles
F32 = mybir.dt.float32
F32R = mybir.dt.float32r

# tile classification codes (host-computed from exp(mask) tiles)
SKIP, PLAIN, MASKED = 0, 1, 2

# content tag: force a fresh NEFF cache key whenever this file changes
# (the jax/neuron compile cache does not key on the embedded BIR)
with open(__file__, "rb") as _f:
    KTAG = hashlib.sha1(_f.read()).hexdigest()[:10]
K_TAG_INT = int(KTAG, 16)


def _build(cls_grid, causal):
    """Build the SPMD program. cls_grid[i][j] in {SKIP, PLAIN, MASKED} for
    scoresT tile (ks_tile i, qs_chunk j). causal=True generates the diagonal
    mask tiles on device (no emaskT input)."""
    nc = bacc.Bacc("TRN2", target_bir_lowering=False)

    # host-packed for contiguous per-partition DMA:
    # x_pre[c, p, kt, s'] = x[b][s = c*CH+s', h = kt*128+p]
    xT = nc.dram_tensor("xT", [NCH, 128, KT, CH], F32R, kind="ExternalInput")
    # w_pre[p, t, kt, o] = w[h = kt*128+p, t*128+o]
    wT = nc.dram_tensor("wT", [128, NT, KT, 128], F32R, kind="ExternalInput")
    AallT = nc.dram_tensor("AallT", [H, 3 * R], F32R, kind="ExternalInput")
    Bpad = nc.dram_tensor("Bpad", [3 * R + 1, NT * 128], F32R, kind="ExternalInput")
    # cache-buster: the PJRT NEFF cache hashes the HLO minus backend_config
    # (where the BIR rides); a tag-dependent input SHAPE forces a new hash.
    DL = (K_TAG_INT % 97) + 1
    dummy = nc.dram_tensor("cachetag", [1, DL], F32, kind="ExternalInput")
    cosT = nc.dram_tensor("cosT", [HD, S], F32, kind="ExternalInput")
    ssT = nc.dram_tensor("ssT", [HD, S], F32, kind="ExternalInput")
    any_masked = any(cls_grid[i][j] == MASKED for i in range(NKS) for j in range(NCH))
    emaskT = None
    if not causal and any_masked:
        emaskT = nc.dram_tensor("emaskT", [S, S], F32R, kind="ExternalInput")
    owT = nc.dram_tensor("owT", [QD, H], F32R, kind="ExternalInput")
    out_p = nc.dram_tensor("out_p", [S, H], F32, kind="ExternalOutput")

    _live_pj = [[i for i in range(NKS) if cls_grid[i][jj] != SKIP] for jj in range(NCH)]
    _need = [max(jj, max(_live_pj[jj]) // (CH // 128)) for jj in range(NCH)]
    QCH_BUFS = max(2, max(_need[jj] - jj for jj in range(NCH)) + 1)

    with tile.TileContext(nc) as tc:
        from concourse.masks import make_identity
        with tc.tile_pool(name="consts", bufs=1) as consts, \
             tc.tile_pool(name="persist", bufs=1) as persist, \
             tc.tile_pool(name="qch", bufs=QCH_BUFS) as qch_pool, \
             tc.tile_pool(name="outp", bufs=2) as outp_pool, \
             tc.tile_pool(name="p1", bufs=2) as p1, \
             tc.tile_pool(name="p1s", bufs=1) as p1s, \
             tc.tile_pool(name="wstream", bufs=3) as wstream, \
             tc.tile_pool(name="p2", bufs=3) as p2, \
             tc.tile_pool(name="stgp", bufs=3) as stgp, \
             tc.tile_pool(name="fin", bufs=1) as fin, \
             tc.tile_pool(name="pp_p", bufs=2, space="PSUM") as pp_p, \
             tc.tile_pool(name="pp_aux", bufs=1, space="PSUM") as pp_aux, \
             tc.tile_pool(name="pp_s", bufs=2, space="PSUM") as pp_s, \
             tc.tile_pool(name="pp_big", bufs=2, space="PSUM") as pp_big, \
             tc.tile_pool(name="pp_sum", bufs=1, space="PSUM") as pp_sum:

            # ---- constants (tiny DMAs first on sync queue) ----
            Aall_sb = consts.tile([128, KT, 3 * R], F32R, tag="Aall_sb")
            nc.sync.dma_start(out=Aall_sb, in_=AallT.rearrange("(kt p) r -> p kt r", p=128))
            Bpad_sb = consts.tile([3 * R + 1, NT * 128], F32R, tag="Bpad_sb")
            nc.sync.dma_start(out=Bpad_sb, in_=Bpad[:, :])
            dummy_sb = consts.tile([1, 128], F32, tag="dummy_sb")
            nc.sync.dma_start(out=dummy_sb[:, 0:DL], in_=dummy[:, :])
            ones_row_f = consts.tile([1, CH], F32, tag="ones_row_f")
            nc.vector.memset(ones_row_f, 1.0)
            ones_row = consts.tile([1, CH], F32R, tag="ones_row")
            nc.vector.tensor_copy(out=ones_row, in_=ones_row_f)
            ones_col_f = consts.tile([128, 1], F32, tag="ones_col_f")
            nc.vector.memset(ones_col_f, 1.0)
            ones_col = consts.tile([128, 1], F32R, tag="ones_col")
            nc.vector.tensor_copy(out=ones_col, in_=ones_col_f)
            ident = consts.tile([128, 128], F32, tag="ident")
            make_identity(nc, ident)

            # causal diagonal mask tiles: pattern depends only on d = i - 4j
            # m[p, q] = 1.0 if p - q + 128*d <= 0 else 0.0
            diag_masks = []
            if causal:
                for dd in range(CH // 128):
                    mf = consts.tile([128, CH], F32, tag=f"diagf{dd}", name=f"diagf{dd}")
                    nc.gpsimd.memset(mf, 0.0)
                    # keep 0 where p - q + 128*dd > 0 (masked), fill 1 elsewhere
                    nc.gpsimd.affine_select(
                        out=mf, in_=mf,
                        compare_op=mybir.AluOpType.is_gt,
                        fill=1.0,
                        base=128 * dd,
                        channel_multiplier=1,
                        pattern=[[-1, CH]],
                    )
                    mr = consts.tile([128, CH], F32R, tag=f"diag{dd}", name=f"diag{dd}")
                    nc.vector.tensor_copy(out=mr, in_=mf)
                    diag_masks.append(mr)

            # ---- persistent tiles ----
            kT_full = persist.tile([128, S], F32R, tag="kT_full")
            v_nat = persist.tile([128, NKS, 128], F32R, tag="v_nat")  # [ks, tile, d]
            ow_sb = persist.tile([128, GQ, H], F32R, tag="ow_sb")

            pending_oproj = None

            def emit_oproj(args):
                cc, outT_ch = args
                for st4 in range(CH // 128):
                    ssl = bass.ds(st4 * 128, 128)
                    dsl = bass.ds((cc * (CH // 128) + st4) * 128, 128)
                    for nch in range(NCH):
                        nsl = bass.ds(nch * CH, CH)
                        pool3 = pp_big if (st4 * NCH + nch) % 2 == 0 else pp_p
                        tag3 = "po3" if (st4 * NCH + nch) % 2 == 0 else "ps_p"
                        ps3 = pool3.tile([128, CH], F32, tag=tag3, name="ps3")
                        for h in range(GQ):
                            nc.tensor.matmul(ps3, outT_ch[h][:, ssl], ow_sb[:, h, nsl],
                                             start=(h == 0), stop=(h == GQ - 1))
                        stg = stgp.tile([128, CH], F32, tag="stg")
                        if (st4 * NCH + nch) % 2 == 0:
                            nc.scalar.activation(out=stg, in_=ps3,
                                                 func=mybir.ActivationFunctionType.Copy)
                            nc.sync.dma_start(out=out_p[dsl, nsl], in_=stg)
                        else:
                            nc.vector.tensor_copy(out=stg, in_=ps3)
                            nc.gpsimd.dma_start(out=out_p[dsl, nsl], in_=stg)

            # per-chunk prerequisite: attention(j) can only run after the
            # proj chunk that produces its highest live k/v tile (and its own
            # q chunk). For causal grids need(j) == j (fully fused pipeline);
            # for dense masks all attention lands after the last proj chunk.
            live_per_j = [[i for i in range(NKS) if cls_grid[i][jj] != SKIP]
                          for jj in range(NCH)]
            masked_per_j = [[i for i in range(NKS) if cls_grid[i][jj] == MASKED]
                            for jj in range(NCH)]
            need = [max(jj, max(live_per_j[jj]) // (CH // 128))
                    for jj in range(NCH)]

            q_chunks = {}

            def emit_proj(c, flush=None):
                sl = bass.ds(c * CH, CH)
                x_c = p1s.tile([128, KT, CH], F32R, tag="x_c", name=f"x_{c}")
                for kq in range(4):
                    nc.sync.dma_start(out=x_c[:, bass.ds(kq * 4, 4), :],
                                      in_=xT[c, :, bass.ds(kq * 4, 4), :])
                cos_c = p1s.tile([128, CH], F32, tag="cos_c", name=f"cos_{c}")
                nc.sync.dma_start(out=cos_c, in_=cosT[:, sl])
                ss_c = p1s.tile([128, CH], F32, tag="ss_c", name=f"ss_{c}")
                nc.sync.dma_start(out=ss_c, in_=ssT[:, sl])

                if flush is not None:
                    emit_oproj(flush)

                # LoRA mid [49, CH] (last row = ones for the bias fold)
                ps_mid = pp_aux.tile([3 * R, CH], F32, tag="aux", name="ps_mid")
                for kt in range(KT):
                    nc.tensor.matmul(ps_mid, Aall_sb[:, kt, :], x_c[:, kt, :],
                                     start=(kt == 0), stop=(kt == KT - 1))
                mid_sb = p1s.tile([3 * R + 1, CH], F32R, tag="mid_sb", name=f"mid_{c}")
                nc.vector.tensor_copy(out=mid_sb[0:3 * R, :], in_=ps_mid)
                nc.sync.dma_start(out=mid_sb[3 * R:3 * R + 1, :], in_=ones_row)

                q_ch = [qch_pool.tile([128, CH], F32R, tag=f"qch{h}", name=f"qch{h}_{c}")
                        for h in range(GQ)]
                q_chunks[c] = q_ch
                for t in range(NT):
                    tsl = bass.ds(t * 128, 128)
                    w_t = wstream.tile([128, KT, 128], F32R, tag="w_t", name=f"w_{c}_{t}")
                    w_eng = nc.scalar if (c == 0 and t < 3) else nc.sync
                    w_eng.dma_start(out=w_t, in_=wT[:, t, :, :])
                    ps_p = pp_p.tile([128, CH], F32, tag="ps_p")
                    for kt in range(KT):
                        nc.tensor.matmul(ps_p, w_t[:, kt, :], x_c[:, kt, :],
                                         start=(kt == 0), stop=False)
                    nc.tensor.matmul(ps_p, Bpad_sb[:, tsl], mid_sb,
                                     start=False, stop=True)
                    if t == NT - 1:   # v: no rope; transpose to v_nat
                        vT_c = p1.tile([128, CH], F32, tag="vT_c")
                        nc.scalar.activation(out=vT_c, in_=ps_p,
                                             func=mybir.ActivationFunctionType.Copy)
                        for i4 in range(CH // 128):
                            i = c * (CH // 128) + i4
                            ps_t = pp_aux.tile([128, 128], F32, tag="aux", name="ps_t")
                            nc.tensor.transpose(ps_t, vT_c[:, bass.ds(i4 * 128, 128)], ident)
                            nc.vector.tensor_copy(out=v_nat[:, i, :], in_=ps_t)
                    else:
                        raw = p1.tile([128, CH], F32, tag="raw")
                        nc.scalar.activation(out=raw, in_=ps_p,
                                             func=mybir.ActivationFunctionType.Copy)
                        # rope: dst = raw*cos + swap(raw)*ss
                        sw = p1.tile([128, CH], F32, tag="sw")
                        nc.sync.dma_start(out=sw[0:64, :], in_=raw[64:128, :])
                        nc.sync.dma_start(out=sw[64:128, :], in_=raw[0:64, :])
                        nc.vector.tensor_mul(sw, sw, ss_c)
                        dst = q_ch[t] if t < GQ else kT_full[:, sl]
                        nc.vector.tensor_mul(dst, raw, cos_c)
                        nc.vector.tensor_add(dst, dst, sw)

                if c == 0:
                    # o_w load: after chunk-0 weight DMAs, on the gpsimd
                    # queue; needed only by the first o_proj
                    nc.gpsimd.dma_start(out=ow_sb,
                                        in_=owT.rearrange("(g p) n -> p g n", p=128))

            def emit_attention(j):
                sl = bass.ds(j * CH, CH)
                live = live_per_j[j]
                masked = masked_per_j[j]
                msk = {}
                stream_masks = False
                if causal:
                    for i in masked:
                        msk[i] = diag_masks[i - 4 * j]
                elif masked and len(masked) <= 6:
                    m_sb = p2.tile([128, len(masked), CH], F32R, tag="m_sb",
                                   name=f"msb_{j}", bufs=2)
                    for mi, i in enumerate(masked):
                        nc.gpsimd.dma_start(out=m_sb[:, mi, :],
                                            in_=emaskT[bass.ds(i * 128, 128), sl])
                        msk[i] = m_sb[:, mi, :]
                elif masked:
                    stream_masks = True

                outT_ch = [outp_pool.tile([128, CH], F32R, tag=f"outT{h}",
                                          name=f"outT{h}_{j}") for h in range(GQ)]
                PIPE = 2
                q_ch = q_chunks[j]
                for h in range(GQ):
                    qh = q_ch[h]
                    ps_o = pp_big.tile([128, CH], F32, tag="po3", name="ps_o")
                    ps_sum = pp_sum.tile([1, CH], F32, tag="ps_sum")
                    n_live = len(live)
                    attns = {}

                    def emit_qk_exp(n):
                        i = live[n]
                        ps_s = pp_s.tile([128, CH], F32, tag="ps_s")
                        nc.tensor.matmul(ps_s, kT_full[:, bass.ds(i * 128, 128)],
                                         qh, start=True, stop=True)
                        attn = p2.tile([128, CH], F32R, tag="attn")
                        nc.scalar.activation(out=attn, in_=ps_s,
                                             func=mybir.ActivationFunctionType.Exp,
                                             scale=float(ATTN_SCALE))
                        if stream_masks and i in masked:
                            mt = p2.tile([128, CH], F32R, tag="m_tile", bufs=4,
                                         name=f"mt_{j}_{h}_{i}")
                            nc.gpsimd.dma_start(out=mt,
                                                in_=emaskT[bass.ds(i * 128, 128), sl])
                            nc.vector.tensor_mul(attn, attn, mt)
                        elif i in msk:
                            nc.vector.tensor_mul(attn, attn, msk[i])
                        attns[n] = attn

                    for n in range(min(PIPE, n_live)):
                        emit_qk_exp(n)
                    for n in range(n_live):
                        if n + PIPE < n_live:
                            emit_qk_exp(n + PIPE)
                        i = live[n]
                        attn = attns.pop(n)
                        first, last = (n == 0), (n == n_live - 1)
                        nc.tensor.matmul(ps_o, v_nat[:, i, :], attn,
                                         start=first, stop=last)
                        nc.tensor.matmul(ps_sum, ones_col, attn,
                                         start=first, stop=last)
                    # finalize: DVE + gpsimd only (no PE involvement)
                    recip_row = fin.tile([1, CH], F32, tag="recip_row")
                    nc.vector.reciprocal_approx_fast(out=recip_row, in_=ps_sum)
                    bcast = fin.tile([128, CH], F32, tag="bcast")
                    nc.gpsimd.partition_broadcast(bcast, recip_row)
                    nc.vector.tensor_mul(outT_ch[h], ps_o, bcast)
                return outT_ch

            for c in range(NCH):
                emit_proj(c, flush=pending_oproj)
                pending_oproj = None
                for j in range(NCH):
                    if need[j] == c:
                        if pending_oproj is not None:
                            emit_oproj(pending_oproj)
                            pending_oproj = None
                        pending_oproj = (j, emit_attention(j))

            if pending_oproj is not None:
                emit_oproj(pending_oproj)
                pending_oproj = None

    nc.finalize()
    return nc


_cache = {}


def _get_program(key, cls_grid, causal):
    if key not in _cache:
        _cache[key] = _build(cls_grid, causal)
    return _cache[key]


def _classify(em_t):
    """em_t: exp(mask).T [S, S] (ks, qs). Returns tuple-of-tuples class grid
    [NKS][NCH]."""
    grid = []
    for i in range(NKS):
        row = []
        for j in range(NCH):
            t = em_t[i * 128:(i + 1) * 128, j * CH:(j + 1) * CH]
            mx = t.max()
            mn = t.min()
            if mx == 0.0:
                row.append(SKIP)
            elif mn == 1.0 and mx == 1.0:
                row.append(PLAIN)
            else:
                row.append(MASKED)
        grid.append(tuple(row))
    return tuple(grid)


def _causal_grid():
    g = []
    for i in range(NKS):
        row = []
        for j in range(NCH):
            if i >= 4 * j + 4:
                row.append(SKIP)
            elif i >= 4 * j:
                row.append(MASKED)
            else:
                row.append(PLAIN)
        g.append(tuple(row))
    return tuple(g)


def _is_exact_causal(emaskT_b):
    """True iff exp(mask).T's diagonal band is exactly the causal 0/1
    pattern (off-band is covered by the grid comparison)."""
    p = np.arange(128)[:, None]
    for jj in range(NCH):
        for i in range(4 * jj, 4 * jj + 4):
            t = emaskT_b[i * 128:(i + 1) * 128, jj * CH:(jj + 1) * CH]
            d = i - 4 * jj
            q = np.arange(CH)[None, :]
            want = (p - q + 128 * d <= 0).astype(np.float32)
            if not np.array_equal(t, want):
                return False
    return True


def kernel(hidden_states, cos, sin, attention_mask,
           q_w, k_w, v_w, q_b, k_b, v_b,
           q_A, q_B, k_A, k_B, v_A, v_B, o_w):
    f32 = np.float32
    hidden_states = np.ascontiguousarray(hidden_states, dtype=f32)
    cos = np.asarray(cos, dtype=f32)
    sin = np.asarray(sin, dtype=f32)
    mask = np.asarray(attention_mask, dtype=f32)[:, 0]  # [B, S, S]

    # host-side shared prep
    with np.errstate(under="ignore", over="ignore"):
        emask = np.exp(np.minimum(mask, 80.0))  # [B, S, S]; clamp avoids inf
    emaskT = [np.ascontiguousarray(emask[b].T) for b in range(B)]
    grids = [_classify(emaskT[b]) for b in range(B)]
    if grids[0] != grids[1]:
        # classifications must agree across cores (same SPMD program):
        # degrade to "multiply everywhere except both-skip"
        grid = tuple(tuple(MASKED if (grids[0][i][j] != SKIP or grids[1][i][j] != SKIP)
                           else SKIP for j in range(NCH)) for i in range(NKS))
    else:
        grid = grids[0]
    # every qs column needs at least one live tile (else div by zero);
    # fall back to fully dense+masked if any column is empty
    for j in range(NCH):
        if all(grid[i][j] == SKIP for i in range(NKS)):
            grid = tuple(tuple(MASKED for _ in range(NCH)) for _ in range(NKS))
            break

    causal = (grid == _causal_grid()
              and all(_is_exact_causal(emaskT[b]) for b in range(B)))

    nc = _get_program((grid, causal), grid, causal)

    # x_pre[c, p, kt, s'] = x[b][c*CH+s', kt*128+p]
    xT = [np.ascontiguousarray(
        hidden_states[b].reshape(NCH, CH, KT, 128).transpose(0, 3, 2, 1))
        for b in range(B)]
    cosT = [np.ascontiguousarray(cos[b].T) for b in range(B)]
    ss = np.concatenate([-sin[:, :, :HD // 2], sin[:, :, HD // 2:]], axis=-1)  # [B,S,HD]
    ssT = [np.ascontiguousarray(ss[b].T) for b in range(B)]

    AallT = np.ascontiguousarray(np.concatenate([q_A, k_A, v_A], axis=1), dtype=f32)

    in_maps = []
    for c in range(NCORES):
        b, g = divmod(c, KVH)
        qsl = slice(QD * g, QD * (g + 1))
        ksl = slice(HD * g, HD * (g + 1))
        w_cat = np.concatenate([q_w[qsl], k_w[ksl], v_w[ksl]], axis=0)  # [768, H]
        # w_pre[p, t, kt, o] = w_cat[t*128+o, kt*128+p]
        wT_c = w_cat.reshape(NT, 128, KT, 128).transpose(3, 0, 2, 1)
        bias_c = np.concatenate([q_b[qsl], k_b[ksl], v_b[ksl]])[None, :]
        Bpad_c = np.zeros((3 * R + 1, NT * 128), f32)
        Bpad_c[0:R, 0:QD] = LORA_SCALE * q_B[:, qsl]
        Bpad_c[R:2 * R, QD:QD + HD] = LORA_SCALE * k_B[:, ksl]
        Bpad_c[2 * R:3 * R, QD + HD:QD + 2 * HD] = LORA_SCALE * v_B[:, ksl]
        Bpad_c[3 * R, :] = bias_c[0]
        owT_c = o_w[:, qsl].T
        m = {
            "xT": xT[b],
            "wT": np.ascontiguousarray(wT_c, dtype=f32),
            "AallT": AallT,
            "Bpad": Bpad_c,
            "cachetag": np.zeros((1, (K_TAG_INT % 97) + 1), f32),
            "cosT": cosT[b],
            "ssT": ssT[b],
            "owT": np.ascontiguousarray(owT_c, dtype=f32),
        }
        if not causal and any(grid[i][j] == MASKED for i in range(NKS) for j in range(NCH)):
            m["emaskT"] = emaskT[b]
        in_maps.append(m)

    res = run_bass_kernel_spmd(nc, in_maps, core_ids=list(range(NCORES)))
    outs = [r["out_p"] for r in res.results]
    full = np.empty((B, S, H), f32)
    for b in range(B):
        full[b] = outs[KVH * b]
        for g in range(1, KVH):
            full[b] += outs[KVH * b + g]
    return full



# revision 12
# speedup vs baseline: 1.3084x; 1.3084x over previous
"""Trainium2 Bass kernel for LoRA-fused QKV + RoPE + GQA causal attention + o_proj.

Problem (hardcoded): B=2, S=2048, H=2048, NH=16, KVH=4, HD=128, R=16.

Sharding: 8 cores = batch(2) x kv-head-group(4). Core c handles batch b=c//4,
kv head g=c%4 (q heads 4g..4g+3). Each core computes its 4 heads' attention and
a partial o_proj ([S,H] partial over its 512 o-dims); host sums 4 partials per
batch.

v2 design vs the fp32r baseline:
- All matmuls in bf16 (1 cycle/row like fp32r, but FWL weight loads and half
  the DMA/SBUF). PSUM accumulation stays fp32. LoRA is folded into W on the
  host (W_eff = W^T + scale*A@B), biases are applied by the scalar engine
  during PSUM evacuation (activation bias=AP per-partition column).
- Everything stays in "transposed space": projections produce qT/kT/vT [d, s],
  scoresT [ks, qs] feeds AV directly, o_proj consumes outT [d, s] stationary.
- Softmax: no max-subtraction; exp on the scalar engine over PAIRS of score
  tiles (one ACTIVATE per 2 PSUM banks halves the ~310ns/instr overhead);
  column sums via an all-ones stationary matmul into PSUM; normalization on
  DVE (reciprocal_approx_fast + gpsimd partition_broadcast + multiply).
- Causal mask applied multiplicatively as exp(mask) with SKIP tiles dropped;
  the diagonal-tile patterns are generated on device as paired bf16 tiles.
- Single fused loop over 4 s-chunks; o_proj of chunk j flushes at the start of
  chunk j+1; attention is software-pipelined 2 score-pairs deep across head
  boundaries so PE never waits on exp.
- Weights resident in SBUF (loaded once); x/cos/sin double-buffered and
  prefetched a chunk ahead; initial DMAs split across queues so the first
  matmul starts ~2us in; output tiles stream back round-robin on 4 queues.
"""

import hashlib
import numpy as np
import ml_dtypes

import concourse.bass as bass
import concourse.mybir as mybir
import concourse.tile as tile
from concourse import bacc
from concourse.bass_utils import run_bass_kernel_spmd

B, S, H = 2, 2048, 2048
NH, KVH, HD = 16, 4, 128
R = 16
LORA_SCALE = 32.0 / 16.0
ATTN_SCALE = HD ** -0.5

NCORES = 8
GQ = NH // KVH          # 4 q heads per core
NT = GQ + 2             # 6 projection tiles: 4 q heads, 1 k, 1 v
QD = GQ * HD            # 512
CH = 512                # s-chunk width
NCH = S // CH           # 4 s-chunks
KT = H // 128           # 16 contraction k-tiles
NKS = S // 128          # 16 ks tiles
F32 = mybir.dt.float32
F32R = mybir.dt.float32r
BF16 = mybir.dt.bfloat16
NPBF16 = ml_dtypes.bfloat16

# tile classification codes (host-computed from exp(mask) tiles)
SKIP, PLAIN, MASKED = 0, 1, 2

# content tag: force a fresh NEFF cache key whenever this file changes
with open(__file__, "rb") as _f:
    KTAG = hashlib.sha1(_f.read()).hexdigest()[:10]
K_TAG_INT = int(KTAG, 16)


def _build(cls_grid, causal):
    """Build the SPMD program. cls_grid[i][j] in {SKIP, PLAIN, MASKED} for
    scoresT tile (ks_tile i, qs_chunk j). causal=True generates the diagonal
    mask tiles on device (no emaskT input)."""
    nc = bacc.Bacc("TRN2", target_bir_lowering=False)

    # host-packed for contiguous per-partition DMA:
    # x_pre[c, p, kt, s'] = x[b][s = c*CH+s', h = kt*128+p]  (bf16)
    xT = nc.dram_tensor("xT", [NCH, 128, KT, CH], BF16, kind="ExternalInput")
    # w_pre[p, t, kt, o] = w_eff[h = kt*128+p, t*128+o]  (bf16, LoRA folded)
    wT = nc.dram_tensor("wT", [128, NT, KT, 128], BF16, kind="ExternalInput")
    # [:, 0:NT] plain bias columns; [:, NT:2*NT] partition-swapped (rotate-half)
    biasT = nc.dram_tensor("biasT", [128, 2 * NT], F32, kind="ExternalInput")
    # cache-buster: the PJRT NEFF cache hashes the HLO minus backend_config
    DL = (K_TAG_INT % 97) + 1
    dummy = nc.dram_tensor("cachetag", [1, DL], F32, kind="ExternalInput")
    cosT = nc.dram_tensor("cosT", [HD, S], BF16, kind="ExternalInput")
    ssT = nc.dram_tensor("ssT", [HD, S], BF16, kind="ExternalInput")
    any_masked = any(cls_grid[i][j] == MASKED for i in range(NKS) for j in range(NCH))
    emaskT = None
    if not causal and any_masked:
        emaskT = nc.dram_tensor("emaskT", [S, S], BF16, kind="ExternalInput")
    owT = nc.dram_tensor("owT", [QD, H], BF16, kind="ExternalInput")
    out_p = nc.dram_tensor("out_p", [S, H], BF16, kind="ExternalOutput")

    live_per_j = [[i for i in range(NKS) if cls_grid[i][jj] != SKIP]
                  for jj in range(NCH)]
    masked_per_j = [[i for i in range(NKS) if cls_grid[i][jj] == MASKED]
                    for jj in range(NCH)]
    need = [max(jj, max(live_per_j[jj]) // (CH // 128)) for jj in range(NCH)]
    QCH_BUFS = max(2, max(need[jj] - jj for jj in range(NCH)) + 1)

    with tile.TileContext(nc) as tc:
        from concourse.masks import make_identity
        with tc.tile_pool(name="consts", bufs=1) as consts, \
             tc.tile_pool(name="persist", bufs=1) as persist, \
             tc.tile_pool(name="qch", bufs=QCH_BUFS) as qch_pool, \
             tc.tile_pool(name="outp", bufs=2) as outp_pool, \
             tc.tile_pool(name="p1", bufs=2) as p1, \
             tc.tile_pool(name="xch", bufs=2) as xch_pool, \
             tc.tile_pool(name="att", bufs=3) as att_pool, \
             tc.tile_pool(name="stgp", bufs=4) as stgp, \
             tc.tile_pool(name="fin", bufs=2) as fin, \
             tc.tile_pool(name="pp_pair", bufs=2, space="PSUM") as pp_pair, \
             tc.tile_pool(name="pp_o", bufs=2, space="PSUM") as pp_o, \
             tc.tile_pool(name="pp_sum", bufs=1, space="PSUM") as pp_sum, \
             tc.tile_pool(name="pp_t", bufs=1, space="PSUM") as pp_t:

            # ---- chunk-0 inputs first: x pieces split across queues ----
            x_tiles = {}

            def emit_x_dma(c):
                x_c = xch_pool.tile([128, KT, CH], BF16, tag="x_c", name=f"x_{c}")
                x_tiles[c] = x_c
                qs = [nc.sync, nc.scalar, nc.sync, nc.scalar]
                for kq in range(4):
                    qs[kq].dma_start(out=x_c[:, bass.ds(kq * 4, 4), :],
                                     in_=xT[c, :, bass.ds(kq * 4, 4), :])

            cs_tiles = {}

            def emit_cs_dma(c):
                sl = bass.ds(c * CH, CH)
                cos_c = xch_pool.tile([128, CH], BF16, tag="cos_c", name=f"cos_{c}")
                nc.sync.dma_start(out=cos_c, in_=cosT[:, sl])
                ss_c = xch_pool.tile([128, CH], BF16, tag="ss_c", name=f"ss_{c}")
                nc.sync.dma_start(out=ss_c, in_=ssT[:, sl])
                cs_tiles[c] = (cos_c, ss_c)

            emit_x_dma(0)
            emit_cs_dma(0)

            # ---- weights resident (q0 first so proj can start ASAP) ----
            w_sb = persist.tile([128, NT, KT, 128], BF16, tag="w_sb")
            w_qs = [nc.gpsimd, nc.sync, nc.gpsimd, nc.sync, nc.gpsimd, nc.sync]
            for t in range(NT):
                w_qs[t].dma_start(out=w_sb[:, t, :, :], in_=wT[:, t, :, :])
            bias_sb = consts.tile([128, 2 * NT], F32, tag="bias_sb")
            nc.gpsimd.dma_start(out=bias_sb, in_=biasT[:, :])
            dummy_sb = consts.tile([1, 128], F32, tag="dummy_sb")
            nc.gpsimd.dma_start(out=dummy_sb[:, 0:DL], in_=dummy[:, :])

            # ---- small constants ----
            ones_col_f = consts.tile([128, 1], F32, tag="ones_col_f")
            nc.vector.memset(ones_col_f, 1.0)
            ones_col = consts.tile([128, 1], BF16, tag="ones_col")
            nc.vector.tensor_copy(out=ones_col, in_=ones_col_f)
            ident_f = consts.tile([128, 128], F32, tag="ident_f")
            make_identity(nc, ident_f)
            identr = consts.tile([128, 128], F32R, tag="identr")
            nc.vector.tensor_copy(out=identr, in_=ident_f)

            # causal diagonal mask tiles, paired: pattern depends on d = i - 4j
            # m_d[p, q] = 1.0 if p - q + 128*d <= 0 else 0.0
            diag_pairs = []
            if causal:
                scratch = consts.tile([128, CH], F32, tag="diag_scratch")
                for pp in range(2):
                    mp = consts.tile([128, 2, CH], BF16, tag=f"diagp{pp}",
                                     name=f"diagp{pp}")
                    for half in range(2):
                        dd = 2 * pp + half
                        nc.gpsimd.memset(scratch, 0.0)
                        nc.gpsimd.affine_select(
                            out=scratch, in_=scratch,
                            compare_op=mybir.AluOpType.is_gt,
                            fill=1.0,
                            base=128 * dd,
                            channel_multiplier=1,
                            pattern=[[-1, CH]],
                        )
                        nc.vector.tensor_copy(out=mp[:, half, :], in_=scratch)
                    diag_pairs.append(mp)

            # ow after the w tiles on gpsimd (first needed by o_proj of chunk 0,
            # which runs during chunk 1)
            ow_sb = persist.tile([128, GQ, H], BF16, tag="ow_sb")
            nc.gpsimd.dma_start(out=ow_sb, in_=owT.rearrange("(g p) n -> p g n", p=128))

            # prefetch chunk 1 inputs behind chunk 0's
            if NCH > 1:
                emit_x_dma(1)
                emit_cs_dma(1)

            # ---- persistent tiles ----
            kT_full = persist.tile([128, S], BF16, tag="kT_full")
            v_nat = persist.tile([128, NKS, 128], BF16, tag="v_nat")  # [ks, tile, d]

            out_dma_q = [nc.sync, nc.gpsimd, nc.scalar]
            out_dma_n = [0]

            def emit_oproj(args):
                cc, outT_ch = args
                for st4 in range(CH // 128):
                    ssl = bass.ds(st4 * 128, 128)
                    dsl = bass.ds((cc * (CH // 128) + st4) * 128, 128)
                    for nch in range(NCH):
                        nsl = bass.ds(nch * CH, CH)
                        ps3 = pp_pair.tile([128, 2, CH], F32, tag="pair", name="ps3")
                        half = (st4 * NCH + nch) % 2
                        for h in range(GQ):
                            nc.tensor.matmul(ps3[:, half, :], outT_ch[h][:, ssl],
                                             ow_sb[:, h, nsl],
                                             start=(h == 0), stop=(h == GQ - 1))
                        stg = stgp.tile([128, CH], BF16, tag="stg")
                        if half == 0:
                            nc.vector.tensor_copy(out=stg, in_=ps3[:, 0, :])
                        else:
                            nc.scalar.activation(out=stg, in_=ps3[:, 1, :],
                                                 func=mybir.ActivationFunctionType.Copy)
                        q = out_dma_q[out_dma_n[0] % 3]
                        out_dma_n[0] += 1
                        q.dma_start(out=out_p[dsl, nsl], in_=stg)

            q_chunks = {}

            def emit_proj(c):
                x_c = x_tiles[c]
                cos_c, ss_c = cs_tiles[c]
                sl = bass.ds(c * CH, CH)

                q_ch = [qch_pool.tile([128, CH], BF16, tag=f"qch{h}", name=f"qch{h}_{c}")
                        for h in range(GQ)]
                q_chunks[c] = q_ch

                pending_v = []

                def do_rope_or_v(t, ps, half):
                    bias_col = bias_sb[:, t:t + 1]
                    bias_sw = bias_sb[:, NT + t:NT + t + 1]
                    if t == NT - 1:   # v: bias only; transpose to v_nat (deferred)
                        vT_c = p1.tile([128, CH], F32R, tag="vT_c")
                        nc.vector.tensor_scalar_add(vT_c, ps[:, half, :], bias_col)
                        pending_v.append(vT_c)
                    else:
                        raw = p1.tile([128, CH], BF16, tag="raw")
                        nc.scalar.activation(out=raw, in_=ps[:, half, :],
                                             func=mybir.ActivationFunctionType.Copy)
                        # rope: dst = (raw+b)*cos + (swap(raw)+swap(b))*ss
                        sw = p1.tile([128, CH], BF16, tag="sw")
                        nc.gpsimd.dma_start(out=sw[0:64, :], in_=raw[64:128, :])
                        nc.gpsimd.dma_start(out=sw[64:128, :], in_=raw[0:64, :])
                        nc.vector.scalar_tensor_tensor(
                            out=sw, in0=sw, scalar=bias_sw, in1=ss_c,
                            op0=mybir.AluOpType.add, op1=mybir.AluOpType.mult)
                        dst = q_ch[t] if t < GQ else kT_full[:, sl]
                        nc.vector.scalar_tensor_tensor(
                            out=dst, in0=raw, scalar=bias_col, in1=cos_c,
                            op0=mybir.AluOpType.add, op1=mybir.AluOpType.mult)
                        nc.vector.tensor_add(dst, dst, sw)

                def flush_v(c):
                    for vT_c in pending_v:
                        for i4 in range(CH // 128):
                            i = c * (CH // 128) + i4
                            ps_t = pp_t.tile([128, 128], F32R, tag="ps_t", name="ps_t")
                            nc.tensor.transpose(ps_t, vT_c[:, bass.ds(i4 * 128, 128)],
                                                identr)
                            nc.vector.tensor_copy(out=v_nat[:, i, :], in_=ps_t)
                    pending_v.clear()

                # t order: q0, q1, k, v, q2 (flush v transposes), q3
                t_list = [0, 1, GQ, NT - 1, 2, 3]
                for n, t in enumerate(t_list):
                    ps = pp_pair.tile([128, 2, CH], F32, tag="pair", name="ps_p")
                    half = n % 2
                    for kt in range(KT):
                        nc.tensor.matmul(ps[:, half, :], w_sb[:, t, kt, :],
                                         x_c[:, kt, :],
                                         start=(kt == 0), stop=(kt == KT - 1))
                    do_rope_or_v(t, ps, half)
                    if t == 2:
                        flush_v(c)
                flush_v(c)

            def emit_attention(j):
                """Emits the attention for chunk j with a 2-pair software
                pipeline across head boundaries."""
                sl = bass.ds(j * CH, CH)
                live = live_per_j[j]
                masked = set(masked_per_j[j])
                # build pair list: (i0, i1 or None)
                pairs = []
                for n in range(0, len(live), 2):
                    i0 = live[n]
                    i1 = live[n + 1] if n + 1 < len(live) else None
                    pairs.append((i0, i1))

                outT_ch = [outp_pool.tile([128, CH], BF16, tag=f"outT{h}",
                                          name=f"outT{h}_{j}") for h in range(GQ)]
                q_ch = q_chunks[j]

                # global pipeline over (head, pair)
                work = [(h, p) for h in range(GQ) for p in pairs]
                NP = len(work)
                attns = {}

                def emit_qk_exp(n):
                    h, (i0, i1) = work[n]
                    qh = q_ch[h]
                    pr = pp_pair.tile([128, 2, CH], F32, tag="pair", name="pr")
                    nc.tensor.matmul(pr[:, 0, :], kT_full[:, bass.ds(i0 * 128, 128)],
                                     qh, start=True, stop=True)
                    if i1 is not None:
                        nc.tensor.matmul(pr[:, 1, :],
                                         kT_full[:, bass.ds(i1 * 128, 128)],
                                         qh, start=True, stop=True)
                    attn = att_pool.tile([128, 2, CH], BF16, tag="attn")
                    src = pr if i1 is not None else pr[:, 0:1, :]
                    dst = attn if i1 is not None else attn[:, 0:1, :]
                    nc.scalar.activation(out=dst, in_=src,
                                         func=mybir.ActivationFunctionType.Exp,
                                         scale=float(ATTN_SCALE))
                    pair_masked = (i0 in masked) or (i1 in masked)
                    if pair_masked:
                        if causal:
                            # diag tiles are i = 4j + d; pairs aligned (d0,d1),(d2,d3)
                            dd = i0 - 4 * j
                            nc.vector.tensor_mul(attn, attn, diag_pairs[dd // 2])
                        else:
                            mt = att_pool.tile([128, 2, CH], BF16, tag="m_tile",
                                               bufs=3, name=f"mt_{j}_{h}_{i0}")
                            nc.gpsimd.dma_start(out=mt[:, 0, :],
                                                in_=emaskT[bass.ds(i0 * 128, 128), sl])
                            if i1 is not None:
                                nc.gpsimd.dma_start(
                                    out=mt[:, 1, :],
                                    in_=emaskT[bass.ds(i1 * 128, 128), sl])
                            msl = attn if i1 is not None else attn[:, 0:1, :]
                            mm = mt if i1 is not None else mt[:, 0:1, :]
                            nc.vector.tensor_mul(msl, msl, mm)
                    attns[n] = attn

                def emit_av(n):
                    h, (i0, i1) = work[n]
                    attn = attns.pop(n)
                    pidx = n % len(pairs)
                    first, last = (pidx == 0), (pidx == len(pairs) - 1)
                    ps_o = st_o[h]
                    ps_sum = st_sum[h]
                    nc.tensor.matmul(ps_o, v_nat[:, i0, :], attn[:, 0, :],
                                     start=first, stop=(last and i1 is None))
                    nc.tensor.matmul(ps_sum, ones_col, attn[:, 0, :],
                                     start=first, stop=(last and i1 is None))
                    if i1 is not None:
                        nc.tensor.matmul(ps_o, v_nat[:, i1, :], attn[:, 1, :],
                                         start=False, stop=last)
                        nc.tensor.matmul(ps_sum, ones_col, attn[:, 1, :],
                                         start=False, stop=last)
                    if last:
                        finalize(h)

                st_o, st_sum = {}, {}

                def start_head(h):
                    st_o[h] = pp_o.tile([128, CH], F32, tag="ps_o", name="ps_o")
                    st_sum[h] = pp_sum.tile([1, CH], F32, tag="ps_sum", name="ps_sum")

                def finalize(h):
                    # DVE + gpsimd only (no PE involvement)
                    recip_row = fin.tile([1, CH], F32, tag="recip_row")
                    nc.vector.reciprocal_approx_fast(out=recip_row, in_=st_sum[h])
                    bcast = fin.tile([128, CH], F32, tag="bcast")
                    nc.gpsimd.partition_broadcast(bcast, recip_row)
                    nc.vector.tensor_mul(outT_ch[h], st_o[h], bcast)

                PIPE = 2
                npairs = len(pairs)
                for n in range(NP):
                    if n % npairs == 0:
                        start_head(work[n][0])
                    emit_qk_exp(n)
                    if n >= PIPE:
                        emit_av(n - PIPE)
                for n in range(max(0, NP - PIPE), NP):
                    emit_av(n)
                return outT_ch

            # ---- main fused loop ----
            pending_oproj = None
            for c in range(NCH):
                if c >= 1 and c + 1 < NCH:
                    emit_x_dma(c + 1)
                    emit_cs_dma(c + 1)
                if pending_oproj is not None:
                    emit_oproj(pending_oproj)
                    pending_oproj = None
                emit_proj(c)
                for j in range(NCH):
                    if need[j] == c:
                        if pending_oproj is not None:
                            emit_oproj(pending_oproj)
                            pending_oproj = None
                        pending_oproj = (j, emit_attention(j))

            if pending_oproj is not None:
                emit_oproj(pending_oproj)
                pending_oproj = None

    nc.finalize()
    return nc


_cache = {}


def _get_program(key, cls_grid, causal):
    if key not in _cache:
        _cache[key] = _build(cls_grid, causal)
    return _cache[key]


def _classify(em_t):
    """em_t: exp(mask).T [S, S] (ks, qs). Returns tuple-of-tuples class grid
    [NKS][NCH]."""
    grid = []
    for i in range(NKS):
        row = []
        for j in range(NCH):
            t = em_t[i * 128:(i + 1) * 128, j * CH:(j + 1) * CH]
            mx = t.max()
            mn = t.min()
            if mx == 0.0:
                row.append(SKIP)
            elif mn == 1.0 and mx == 1.0:
                row.append(PLAIN)
            else:
                row.append(MASKED)
        grid.append(tuple(row))
    return tuple(grid)


def _causal_grid():
    g = []
    for i in range(NKS):
        row = []
        for j in range(NCH):
            if i >= 4 * j + 4:
                row.append(SKIP)
            elif i >= 4 * j:
                row.append(MASKED)
            else:
                row.append(PLAIN)
        g.append(tuple(row))
    return tuple(g)


def _is_exact_causal(emaskT_b):
    """True iff exp(mask).T's diagonal band is exactly the causal 0/1
    pattern (off-band is covered by the grid comparison)."""
    p = np.arange(128)[:, None]
    for jj in range(NCH):
        for i in range(4 * jj, 4 * jj + 4):
            t = emaskT_b[i * 128:(i + 1) * 128, jj * CH:(jj + 1) * CH]
            d = i - 4 * jj
            q = np.arange(CH)[None, :]
            want = (p - q + 128 * d <= 0).astype(np.float32)
            if not np.array_equal(t, want):
                return False
    return True


def kernel(hidden_states, cos, sin, attention_mask,
           q_w, k_w, v_w, q_b, k_b, v_b,
           q_A, q_B, k_A, k_B, v_A, v_B, o_w):
    f32 = np.float32
    hidden_states = np.asarray(hidden_states, dtype=f32)
    cos = np.asarray(cos, dtype=f32)
    sin = np.asarray(sin, dtype=f32)
    mask = np.asarray(attention_mask, dtype=f32)[:, 0]  # [B, S, S]

    # host-side shared prep
    with np.errstate(under="ignore", over="ignore"):
        emask = np.exp(np.minimum(mask, 80.0))  # [B, S, S]; clamp avoids inf
    emaskT = [np.ascontiguousarray(emask[b].T) for b in range(B)]
    grids = [_classify(emaskT[b]) for b in range(B)]
    if grids[0] != grids[1]:
        grid = tuple(tuple(MASKED if (grids[0][i][j] != SKIP or grids[1][i][j] != SKIP)
                           else SKIP for j in range(NCH)) for i in range(NKS))
    else:
        grid = grids[0]
    for j in range(NCH):
        if all(grid[i][j] == SKIP for i in range(NKS)):
            grid = tuple(tuple(MASKED for _ in range(NCH)) for _ in range(NKS))
            break

    causal = (grid == _causal_grid()
              and all(_is_exact_causal(emaskT[b]) for b in range(B)))

    nc = _get_program((grid, causal), grid, causal)

    # x_pre[c, p, kt, s'] = x[b][c*CH+s', kt*128+p]
    xT = [np.ascontiguousarray(
        hidden_states[b].reshape(NCH, CH, KT, 128).transpose(0, 3, 2, 1)
        ).astype(NPBF16) for b in range(B)]
    cosT = [np.ascontiguousarray(cos[b].T).astype(NPBF16) for b in range(B)]
    ss = np.concatenate([-sin[:, :, :HD // 2], sin[:, :, HD // 2:]], axis=-1)
    ssT = [np.ascontiguousarray(ss[b].T).astype(NPBF16) for b in range(B)]
    emaskT16 = None

    # effective weights: W_eff[outdim, h] = W[outdim, h] + s*(A @ B).T[outdim, h]
    qw_eff = q_w + LORA_SCALE * (q_A @ q_B).T
    kw_eff = k_w + LORA_SCALE * (k_A @ k_B).T
    vw_eff = v_w + LORA_SCALE * (v_A @ v_B).T

    in_maps = []
    for c in range(NCORES):
        b, g = divmod(c, KVH)
        qsl = slice(QD * g, QD * (g + 1))
        ksl = slice(HD * g, HD * (g + 1))
        w_cat = np.concatenate([qw_eff[qsl], kw_eff[ksl], vw_eff[ksl]], axis=0)
        # w_pre[p, t, kt, o] = w_cat[t*128+o, kt*128+p]
        wT_c = w_cat.reshape(NT, 128, KT, 128).transpose(3, 0, 2, 1)
        bias_cat = np.concatenate([q_b[qsl], k_b[ksl], v_b[ksl]]).astype(f32)
        bias_cols = bias_cat.reshape(NT, 128).T  # [128, NT]
        swap_idx = np.concatenate([np.arange(64, 128), np.arange(0, 64)])
        biasT_c = np.ascontiguousarray(
            np.concatenate([bias_cols, bias_cols[swap_idx]], axis=1))  # [128, 2*NT]
        owT_c = o_w[:, qsl].T
        m = {
            "xT": xT[b],
            "wT": np.ascontiguousarray(wT_c).astype(NPBF16),
            "biasT": biasT_c,
            "cachetag": np.zeros((1, (K_TAG_INT % 97) + 1), f32),
            "cosT": cosT[b],
            "ssT": ssT[b],
            "owT": np.ascontiguousarray(owT_c).astype(NPBF16),
        }
        if not causal and any(grid[i][j] == MASKED for i in range(NKS) for j in range(NCH)):
            if emaskT16 is None:
                emaskT16 = [e.astype(NPBF16) for e in emaskT]
            m["emaskT"] = emaskT16[b]
        in_maps.append(m)

    res = run_bass_kernel_spmd(nc, in_maps, core_ids=list(range(NCORES)))
    outs = [np.asarray(r["out_p"], dtype=f32) for r in res.results]
    full = np.empty((B, S, H), f32)
    for b in range(B):
        full[b] = outs[KVH * b]
        for g in range(1, KVH):
            full[b] += outs[KVH * b + g]
    return full


# revision 17
# speedup vs baseline: 1.3228x; 1.0110x over previous
"""Trainium2 Bass kernel for LoRA-fused QKV + RoPE + GQA causal attention + o_proj.

Problem (hardcoded): B=2, S=2048, H=2048, NH=16, KVH=4, HD=128, R=16.

Sharding: 8 cores = batch(2) x kv-head-group(4). Core c handles batch b=c//4,
kv head g=c%4 (q heads 4g..4g+3). Each core computes its 4 heads' attention and
a partial o_proj ([S,H] partial over its 512 o-dims); host sums 4 partials per
batch.

v2 design vs the fp32r baseline:
- All matmuls in bf16 (1 cycle/row like fp32r, but FWL weight loads and half
  the DMA/SBUF). PSUM accumulation stays fp32. LoRA is folded into W on the
  host (W_eff = W^T + scale*A@B), biases are applied by the scalar engine
  during PSUM evacuation (activation bias=AP per-partition column).
- Everything stays in "transposed space": projections produce qT/kT/vT [d, s],
  scoresT [ks, qs] feeds AV directly, o_proj consumes outT [d, s] stationary.
- Softmax: no max-subtraction; exp on the scalar engine over PAIRS of score
  tiles (one ACTIVATE per 2 PSUM banks halves the ~310ns/instr overhead);
  column sums via an all-ones stationary matmul into PSUM; normalization on
  DVE (reciprocal_approx_fast + gpsimd partition_broadcast + multiply).
- Causal mask applied multiplicatively as exp(mask) with SKIP tiles dropped;
  the diagonal-tile patterns are generated on device as paired bf16 tiles.
- Single fused loop over 4 s-chunks; o_proj of chunk j flushes at the start of
  chunk j+1; attention is software-pipelined 2 score-pairs deep across head
  boundaries so PE never waits on exp.
- Weights resident in SBUF (loaded once); x/cos/sin double-buffered and
  prefetched a chunk ahead; initial DMAs split across queues so the first
  matmul starts ~2us in; output tiles stream back round-robin on 4 queues.
"""

import hashlib
import numpy as np
import ml_dtypes

import concourse.bass as bass
import concourse.mybir as mybir
import concourse.tile as tile
from concourse import bacc
from concourse.bass_utils import run_bass_kernel_spmd

B, S, H = 2, 2048, 2048
NH, KVH, HD = 16, 4, 128
R = 16
LORA_SCALE = 32.0 / 16.0
ATTN_SCALE = HD ** -0.5

NCORES = 8
GQ = NH // KVH          # 4 q heads per core
NT = GQ + 2             # 6 projection tiles: 4 q heads, 1 k, 1 v
QD = GQ * HD            # 512
CH = 512                # s-chunk width
NCH = S // CH           # 4 s-chunks
KT = H // 128           # 16 contraction k-tiles
NKS = S // 128          # 16 ks tiles
F32 = mybir.dt.float32
F32R = mybir.dt.float32r
BF16 = mybir.dt.bfloat16
NPBF16 = ml_dtypes.bfloat16

# tile classification codes (host-computed from exp(mask) tiles)
SKIP, PLAIN, MASKED = 0, 1, 2

# content tag: force a fresh NEFF cache key whenever this file changes
with open(__file__, "rb") as _f:
    KTAG = hashlib.sha1(_f.read()).hexdigest()[:10]
K_TAG_INT = int(KTAG, 16)


def _build(cls_grid, causal):
    """Build the SPMD program. cls_grid[i][j] in {SKIP, PLAIN, MASKED} for
    scoresT tile (ks_tile i, qs_chunk j). causal=True generates the diagonal
    mask tiles on device (no emaskT input)."""
    nc = bacc.Bacc("TRN2", target_bir_lowering=False)

    # host-packed for contiguous per-partition DMA:
    # x_pre[c, p, kt, s'] = x[b][s = c*CH+s', h = kt*128+p]  (bf16)
    xT = nc.dram_tensor("xT", [NCH, 128, KT, CH], BF16, kind="ExternalInput")
    # w_pre[p, t, kt, o] = w_eff[h = kt*128+p, t*128+o]  (bf16, LoRA folded)
    wT = nc.dram_tensor("wT", [128, NT, KT, 128], BF16, kind="ExternalInput")
    # [:, 0:NT] plain bias columns; [:, NT:2*NT] partition-swapped (rotate-half)
    biasT = nc.dram_tensor("biasT", [128, 2 * NT], F32, kind="ExternalInput")
    # cache-buster: the PJRT NEFF cache hashes the HLO minus backend_config
    DL = (K_TAG_INT % 97) + 1
    dummy = nc.dram_tensor("cachetag", [1, DL], F32, kind="ExternalInput")
    cosT = nc.dram_tensor("cosT", [HD, S], BF16, kind="ExternalInput")
    ssT = nc.dram_tensor("ssT", [HD, S], BF16, kind="ExternalInput")
    any_masked = any(cls_grid[i][j] == MASKED for i in range(NKS) for j in range(NCH))
    emaskT = None
    if not causal and any_masked:
        emaskT = nc.dram_tensor("emaskT", [S, S], BF16, kind="ExternalInput")
    owT = nc.dram_tensor("owT", [QD, H], BF16, kind="ExternalInput")
    out_p = nc.dram_tensor("out_p", [S, H], BF16, kind="ExternalOutput")

    live_per_j = [[i for i in range(NKS) if cls_grid[i][jj] != SKIP]
                  for jj in range(NCH)]
    masked_per_j = [[i for i in range(NKS) if cls_grid[i][jj] == MASKED]
                    for jj in range(NCH)]
    need = [max(jj, max(live_per_j[jj]) // (CH // 128)) for jj in range(NCH)]
    QCH_BUFS = max(2, max(need[jj] - jj for jj in range(NCH)) + 1)

    with tile.TileContext(nc) as tc:
        from concourse.masks import make_identity
        with tc.tile_pool(name="consts", bufs=1) as consts, \
             tc.tile_pool(name="persist", bufs=1) as persist, \
             tc.tile_pool(name="qch", bufs=QCH_BUFS) as qch_pool, \
             tc.tile_pool(name="outp", bufs=2) as outp_pool, \
             tc.tile_pool(name="p1", bufs=2) as p1, \
             tc.tile_pool(name="xch", bufs=2) as xch_pool, \
             tc.tile_pool(name="att", bufs=3) as att_pool, \
             tc.tile_pool(name="stgp", bufs=4) as stgp, \
             tc.tile_pool(name="fin", bufs=2) as fin, \
             tc.tile_pool(name="pp_pair", bufs=2, space="PSUM") as pp_pair, \
             tc.tile_pool(name="pp_o", bufs=2, space="PSUM") as pp_o, \
             tc.tile_pool(name="pp_sum", bufs=1, space="PSUM") as pp_sum, \
             tc.tile_pool(name="pp_t", bufs=1, space="PSUM") as pp_t:

            # ---- chunk-0 inputs first: x pieces split across queues ----
            x_tiles = {}

            def emit_x_dma(c, fine=False):
                x_c = xch_pool.tile([128, KT, CH], BF16, tag="x_c", name=f"x_{c}")
                x_tiles[c] = x_c
                if fine:
                    # 16 single-kt pieces: first matmul starts after 0.13MB
                    for kt in range(KT):
                        q = nc.sync if kt % 2 == 0 else nc.gpsimd
                        q.dma_start(out=x_c[:, bass.ds(kt, 1), :],
                                    in_=xT[c, :, bass.ds(kt, 1), :])
                else:
                    qs = [nc.sync, nc.gpsimd, nc.sync, nc.gpsimd]
                    for kq in range(4):
                        qs[kq].dma_start(out=x_c[:, bass.ds(kq * 4, 4), :],
                                         in_=xT[c, :, bass.ds(kq * 4, 4), :])

            cs_tiles = {}

            def emit_cs_dma(c):
                sl = bass.ds(c * CH, CH)
                cos_c = xch_pool.tile([128, CH], BF16, tag="cos_c", name=f"cos_{c}")
                nc.sync.dma_start(out=cos_c, in_=cosT[:, sl])
                ss_c = xch_pool.tile([128, CH], BF16, tag="ss_c", name=f"ss_{c}")
                nc.sync.dma_start(out=ss_c, in_=ssT[:, sl])
                cs_tiles[c] = (cos_c, ss_c)

            # ---- weights resident, all on the scalar queue in t-need order;
            # x chunk 0 finely split on sync/gpsimd in parallel ----
            w_sb = persist.tile([128, NT, KT, 128], BF16, tag="w_sb")
            nc.scalar.dma_start(out=w_sb[:, 0, :, :], in_=wT[:, 0, :, :])
            emit_x_dma(0, fine=True)
            for t in [1, GQ, NT - 1, 2, 3]:
                nc.scalar.dma_start(out=w_sb[:, t, :, :], in_=wT[:, t, :, :])
            emit_cs_dma(0)
            bias_sb = consts.tile([128, 2 * NT], F32, tag="bias_sb")
            nc.gpsimd.dma_start(out=bias_sb, in_=biasT[:, :])
            dummy_sb = consts.tile([1, 128], F32, tag="dummy_sb")
            nc.gpsimd.dma_start(out=dummy_sb[:, 0:DL], in_=dummy[:, :])

            # ---- small constants ----
            ones_col_f = consts.tile([128, 1], F32, tag="ones_col_f")
            nc.vector.memset(ones_col_f, 1.0)
            ones_col = consts.tile([128, 1], BF16, tag="ones_col")
            nc.vector.tensor_copy(out=ones_col, in_=ones_col_f)
            ident_f = consts.tile([128, 128], F32, tag="ident_f")
            make_identity(nc, ident_f)
            identr = consts.tile([128, 128], F32R, tag="identr")
            nc.vector.tensor_copy(out=identr, in_=ident_f)

            # causal diagonal mask tiles, paired: pattern depends on d = i - 4j
            # m_d[p, q] = 1.0 if p - q + 128*d <= 0 else 0.0
            diag_pairs = []
            if causal:
                scratch = consts.tile([128, CH], F32, tag="diag_scratch")
                for pp in range(2):
                    mp = consts.tile([128, 2, CH], BF16, tag=f"diagp{pp}",
                                     name=f"diagp{pp}")
                    for half in range(2):
                        dd = 2 * pp + half
                        nc.gpsimd.memset(scratch, 0.0)
                        nc.gpsimd.affine_select(
                            out=scratch, in_=scratch,
                            compare_op=mybir.AluOpType.is_gt,
                            fill=1.0,
                            base=128 * dd,
                            channel_multiplier=1,
                            pattern=[[-1, CH]],
                        )
                        nc.vector.tensor_copy(out=mp[:, half, :], in_=scratch)
                    diag_pairs.append(mp)

            # prefetch chunk 1 inputs behind chunk 0's
            if NCH > 1:
                emit_x_dma(1)
                emit_cs_dma(1)

            # ow last (first needed by o_proj of chunk 0, after proj of chunk 1)
            ow_sb = persist.tile([128, GQ, H], BF16, tag="ow_sb")
            nc.gpsimd.dma_start(out=ow_sb, in_=owT.rearrange("(g p) n -> p g n", p=128))

            # ---- persistent tiles ----
            kT_full = persist.tile([128, S], BF16, tag="kT_full")
            v_nat = persist.tile([128, NKS, 128], BF16, tag="v_nat")  # [ks, tile, d]

            out_dma_q = [nc.sync, nc.gpsimd]
            out_dma_n = [0]

            def emit_oproj(args):
                cc, outT_ch = args
                for st4 in range(CH // 128):
                    ssl = bass.ds(st4 * 128, 128)
                    dsl = bass.ds((cc * (CH // 128) + st4) * 128, 128)
                    for nch in range(NCH):
                        nsl = bass.ds(nch * CH, CH)
                        ps3 = pp_pair.tile([128, 2, CH], F32, tag="pair", name="ps3")
                        half = (st4 * NCH + nch) % 2
                        for h in range(GQ):
                            nc.tensor.matmul(ps3[:, half, :], outT_ch[h][:, ssl],
                                             ow_sb[:, h, nsl],
                                             start=(h == 0), stop=(h == GQ - 1))
                        stg = stgp.tile([128, CH], BF16, tag="stg")
                        nc.vector.tensor_copy(out=stg, in_=ps3[:, half, :])
                        q = out_dma_q[out_dma_n[0] % 2]
                        out_dma_n[0] += 1
                        q.dma_start(out=out_p[dsl, nsl], in_=stg)

            q_chunks = {}

            def emit_proj(c):
                x_c = x_tiles[c]
                cos_c, ss_c = cs_tiles[c]
                sl = bass.ds(c * CH, CH)

                q_ch = [qch_pool.tile([128, CH], BF16, tag=f"qch{h}", name=f"qch{h}_{c}")
                        for h in range(GQ)]
                q_chunks[c] = q_ch

                pending_v = []

                def do_rope_or_v(t, ps, half):
                    bias_col = bias_sb[:, t:t + 1]
                    bias_sw = bias_sb[:, NT + t:NT + t + 1]
                    if t == NT - 1:   # v: bias only; transpose to v_nat (deferred)
                        vT_c = p1.tile([128, CH], F32R, tag="vT_c")
                        nc.vector.tensor_scalar_add(vT_c, ps[:, half, :], bias_col)
                        pending_v.append(vT_c)
                    else:
                        raw = p1.tile([128, CH], BF16, tag="raw")
                        nc.scalar.activation(out=raw, in_=ps[:, half, :],
                                             func=mybir.ActivationFunctionType.Copy)
                        # rope: dst = (raw+b)*cos + (swap(raw)+swap(b))*ss
                        sw = p1.tile([128, CH], BF16, tag="sw")
                        nc.gpsimd.dma_start(out=sw[0:64, :], in_=raw[64:128, :])
                        nc.gpsimd.dma_start(out=sw[64:128, :], in_=raw[0:64, :])
                        nc.vector.scalar_tensor_tensor(
                            out=sw, in0=sw, scalar=bias_sw, in1=ss_c,
                            op0=mybir.AluOpType.add, op1=mybir.AluOpType.mult)
                        dst = q_ch[t] if t < GQ else kT_full[:, sl]
                        nc.vector.scalar_tensor_tensor(
                            out=dst, in0=raw, scalar=bias_col, in1=cos_c,
                            op0=mybir.AluOpType.add, op1=mybir.AluOpType.mult)
                        nc.vector.tensor_add(dst, dst, sw)

                def flush_v(c):
                    for vT_c in pending_v:
                        for i4 in range(CH // 128):
                            i = c * (CH // 128) + i4
                            ps_t = pp_t.tile([128, 128], F32R, tag="ps_t", name="ps_t")
                            nc.tensor.transpose(ps_t, vT_c[:, bass.ds(i4 * 128, 128)],
                                                identr)
                            nc.vector.tensor_copy(out=v_nat[:, i, :], in_=ps_t)
                    pending_v.clear()

                # t order: q0, q1, k, v, q2 (flush v transposes), q3
                t_list = [0, 1, GQ, NT - 1, 2, 3]
                for n, t in enumerate(t_list):
                    ps = pp_pair.tile([128, 2, CH], F32, tag="pair", name="ps_p")
                    half = n % 2
                    for kt in range(KT):
                        nc.tensor.matmul(ps[:, half, :], w_sb[:, t, kt, :],
                                         x_c[:, kt, :],
                                         start=(kt == 0), stop=(kt == KT - 1))
                    do_rope_or_v(t, ps, half)
                    if t == 2:
                        flush_v(c)
                flush_v(c)

            def emit_attention(j):
                """Emits the attention for chunk j with a 2-pair software
                pipeline across head boundaries."""
                sl = bass.ds(j * CH, CH)
                live = live_per_j[j]
                masked = set(masked_per_j[j])
                # build pair list: (i0, i1 or None)
                pairs = []
                for n in range(0, len(live), 2):
                    i0 = live[n]
                    i1 = live[n + 1] if n + 1 < len(live) else None
                    pairs.append((i0, i1))

                outT_ch = [outp_pool.tile([128, CH], BF16, tag=f"outT{h}",
                                          name=f"outT{h}_{j}") for h in range(GQ)]
                q_ch = q_chunks[j]

                # global pipeline over (head, pair)
                work = [(h, p) for h in range(GQ) for p in pairs]
                NP = len(work)
                attns = {}

                def emit_qk_exp(n):
                    h, (i0, i1) = work[n]
                    qh = q_ch[h]
                    pr = pp_pair.tile([128, 2, CH], F32, tag="pair", name="pr")
                    nc.tensor.matmul(pr[:, 0, :], kT_full[:, bass.ds(i0 * 128, 128)],
                                     qh, start=True, stop=True)
                    if i1 is not None:
                        nc.tensor.matmul(pr[:, 1, :],
                                         kT_full[:, bass.ds(i1 * 128, 128)],
                                         qh, start=True, stop=True)
                    attn = att_pool.tile([128, 2, CH], BF16, tag="attn")
                    src = pr if i1 is not None else pr[:, 0:1, :]
                    dst = attn if i1 is not None else attn[:, 0:1, :]
                    nc.scalar.activation(out=dst, in_=src,
                                         func=mybir.ActivationFunctionType.Exp,
                                         scale=float(ATTN_SCALE))
                    pair_masked = (i0 in masked) or (i1 in masked)
                    if pair_masked:
                        if causal:
                            # diag tiles are i = 4j + d; pairs aligned (d0,d1),(d2,d3)
                            dd = i0 - 4 * j
                            nc.vector.tensor_mul(attn, attn, diag_pairs[dd // 2])
                        else:
                            mt = att_pool.tile([128, 2, CH], BF16, tag="m_tile",
                                               bufs=3, name=f"mt_{j}_{h}_{i0}")
                            nc.gpsimd.dma_start(out=mt[:, 0, :],
                                                in_=emaskT[bass.ds(i0 * 128, 128), sl])
                            if i1 is not None:
                                nc.gpsimd.dma_start(
                                    out=mt[:, 1, :],
                                    in_=emaskT[bass.ds(i1 * 128, 128), sl])
                            msl = attn if i1 is not None else attn[:, 0:1, :]
                            mm = mt if i1 is not None else mt[:, 0:1, :]
                            nc.vector.tensor_mul(msl, msl, mm)
                    attns[n] = attn

                def emit_av(n):
                    h, (i0, i1) = work[n]
                    attn = attns.pop(n)
                    pidx = n % len(pairs)
                    first, last = (pidx == 0), (pidx == len(pairs) - 1)
                    ps_o = st_o[h]
                    ps_sum = st_sum[h]
                    nc.tensor.matmul(ps_o, v_nat[:, i0, :], attn[:, 0, :],
                                     start=first, stop=(last and i1 is None))
                    nc.tensor.matmul(ps_sum, ones_col, attn[:, 0, :],
                                     start=first, stop=(last and i1 is None))
                    if i1 is not None:
                        nc.tensor.matmul(ps_o, v_nat[:, i1, :], attn[:, 1, :],
                                         start=False, stop=last)
                        nc.tensor.matmul(ps_sum, ones_col, attn[:, 1, :],
                                         start=False, stop=last)
                    if last:
                        finalize(h)

                st_o, st_sum = {}, {}

                def start_head(h):
                    st_o[h] = pp_o.tile([128, CH], F32, tag="ps_o", name="ps_o")
                    st_sum[h] = pp_sum.tile([1, CH], F32, tag="ps_sum", name="ps_sum")

                def finalize(h):
                    # DVE + gpsimd only (no PE involvement)
                    recip_row = fin.tile([1, CH], F32, tag="recip_row")
                    nc.vector.reciprocal_approx_fast(out=recip_row, in_=st_sum[h])
                    bcast = fin.tile([128, CH], F32, tag="bcast")
                    nc.gpsimd.partition_broadcast(bcast, recip_row)
                    nc.vector.tensor_mul(outT_ch[h], st_o[h], bcast)

                PIPE = 2
                npairs = len(pairs)
                for n in range(NP):
                    if n % npairs == 0:
                        start_head(work[n][0])
                    emit_qk_exp(n)
                    if n >= PIPE:
                        emit_av(n - PIPE)
                for n in range(max(0, NP - PIPE), NP):
                    emit_av(n)
                return outT_ch

            # ---- main fused loop: x(c+1) prefetch -> proj(c) -> o_proj(c-1)
            # flush (outT finalize gets proj-length slack) -> attention(c) ----
            pending_oproj = None
            for c in range(NCH):
                if c >= 1 and c + 1 < NCH:
                    emit_x_dma(c + 1)
                    emit_cs_dma(c + 1)
                emit_proj(c)
                if pending_oproj is not None:
                    emit_oproj(pending_oproj)
                    pending_oproj = None
                for j in range(NCH):
                    if need[j] == c:
                        if pending_oproj is not None:
                            emit_oproj(pending_oproj)
                            pending_oproj = None
                        pending_oproj = (j, emit_attention(j))

            if pending_oproj is not None:
                emit_oproj(pending_oproj)
                pending_oproj = None

    nc.finalize()
    return nc


_cache = {}


def _get_program(key, cls_grid, causal):
    if key not in _cache:
        _cache[key] = _build(cls_grid, causal)
    return _cache[key]


def _classify(em_t):
    """em_t: exp(mask).T [S, S] (ks, qs). Returns tuple-of-tuples class grid
    [NKS][NCH]."""
    grid = []
    for i in range(NKS):
        row = []
        for j in range(NCH):
            t = em_t[i * 128:(i + 1) * 128, j * CH:(j + 1) * CH]
            mx = t.max()
            mn = t.min()
            if mx == 0.0:
                row.append(SKIP)
            elif mn == 1.0 and mx == 1.0:
                row.append(PLAIN)
            else:
                row.append(MASKED)
        grid.append(tuple(row))
    return tuple(grid)


def _causal_grid():
    g = []
    for i in range(NKS):
        row = []
        for j in range(NCH):
            if i >= 4 * j + 4:
                row.append(SKIP)
            elif i >= 4 * j:
                row.append(MASKED)
            else:
                row.append(PLAIN)
        g.append(tuple(row))
    return tuple(g)


def _is_exact_causal(emaskT_b):
    """True iff exp(mask).T's diagonal band is exactly the causal 0/1
    pattern (off-band is covered by the grid comparison)."""
    p = np.arange(128)[:, None]
    for jj in range(NCH):
        for i in range(4 * jj, 4 * jj + 4):
            t = emaskT_b[i * 128:(i + 1) * 128, jj * CH:(jj + 1) * CH]
            d = i - 4 * jj
            q = np.arange(CH)[None, :]
            want = (p - q + 128 * d <= 0).astype(np.float32)
            if not np.array_equal(t, want):
                return False
    return True


def kernel(hidden_states, cos, sin, attention_mask,
           q_w, k_w, v_w, q_b, k_b, v_b,
           q_A, q_B, k_A, k_B, v_A, v_B, o_w):
    f32 = np.float32
    hidden_states = np.asarray(hidden_states, dtype=f32)
    cos = np.asarray(cos, dtype=f32)
    sin = np.asarray(sin, dtype=f32)
    mask = np.asarray(attention_mask, dtype=f32)[:, 0]  # [B, S, S]

    # host-side shared prep
    with np.errstate(under="ignore", over="ignore"):
        emask = np.exp(np.minimum(mask, 80.0))  # [B, S, S]; clamp avoids inf
    emaskT = [np.ascontiguousarray(emask[b].T) for b in range(B)]
    grids = [_classify(emaskT[b]) for b in range(B)]
    if grids[0] != grids[1]:
        grid = tuple(tuple(MASKED if (grids[0][i][j] != SKIP or grids[1][i][j] != SKIP)
                           else SKIP for j in range(NCH)) for i in range(NKS))
    else:
        grid = grids[0]
    for j in range(NCH):
        if all(grid[i][j] == SKIP for i in range(NKS)):
            grid = tuple(tuple(MASKED for _ in range(NCH)) for _ in range(NKS))
            break

    causal = (grid == _causal_grid()
              and all(_is_exact_causal(emaskT[b]) for b in range(B)))

    nc = _get_program((grid, causal), grid, causal)

    # x_pre[c, p, kt, s'] = x[b][c*CH+s', kt*128+p]
    xT = [np.ascontiguousarray(
        hidden_states[b].reshape(NCH, CH, KT, 128).transpose(0, 3, 2, 1)
        ).astype(NPBF16) for b in range(B)]
    cosT = [np.ascontiguousarray(cos[b].T).astype(NPBF16) for b in range(B)]
    ss = np.concatenate([-sin[:, :, :HD // 2], sin[:, :, HD // 2:]], axis=-1)
    ssT = [np.ascontiguousarray(ss[b].T).astype(NPBF16) for b in range(B)]
    emaskT16 = None

    # effective weights: W_eff[outdim, h] = W[outdim, h] + s*(A @ B).T[outdim, h]
    qw_eff = q_w + LORA_SCALE * (q_A @ q_B).T
    kw_eff = k_w + LORA_SCALE * (k_A @ k_B).T
    vw_eff = v_w + LORA_SCALE * (v_A @ v_B).T

    in_maps = []
    for c in range(NCORES):
        b, g = divmod(c, KVH)
        qsl = slice(QD * g, QD * (g + 1))
        ksl = slice(HD * g, HD * (g + 1))
        w_cat = np.concatenate([qw_eff[qsl], kw_eff[ksl], vw_eff[ksl]], axis=0)
        # w_pre[p, t, kt, o] = w_cat[t*128+o, kt*128+p]
        wT_c = w_cat.reshape(NT, 128, KT, 128).transpose(3, 0, 2, 1)
        bias_cat = np.concatenate([q_b[qsl], k_b[ksl], v_b[ksl]]).astype(f32)
        bias_cols = bias_cat.reshape(NT, 128).T  # [128, NT]
        swap_idx = np.concatenate([np.arange(64, 128), np.arange(0, 64)])
        biasT_c = np.ascontiguousarray(
            np.concatenate([bias_cols, bias_cols[swap_idx]], axis=1))  # [128, 2*NT]
        owT_c = o_w[:, qsl].T
        m = {
            "xT": xT[b],
            "wT": np.ascontiguousarray(wT_c).astype(NPBF16),
            "biasT": biasT_c,
            "cachetag": np.zeros((1, (K_TAG_INT % 97) + 1), f32),
            "cosT": cosT[b],
            "ssT": ssT[b],
            "owT": np.ascontiguousarray(owT_c).astype(NPBF16),
        }
        if not causal and any(grid[i][j] == MASKED for i in range(NKS) for j in range(NCH)):
            if emaskT16 is None:
                emaskT16 = [e.astype(NPBF16) for e in emaskT]
            m["emaskT"] = emaskT16[b]
        in_maps.append(m)

    res = run_bass_kernel_spmd(nc, in_maps, core_ids=list(range(NCORES)))
    outs = [np.asarray(r["out_p"], dtype=f32) for r in res.results]
    full = np.empty((B, S, H), f32)
    for b in range(B):
        full[b] = outs[KVH * b]
        for g in range(1, KVH):
            full[b] += outs[KVH * b + g]
    return full
